# revision 23
# baseline (speedup 1.0000x reference)
"""GCN layer (symmetric-normalized, self-loops) on 8 Trainium2 NeuronCores.

out[d] = sum_{e:(s,d)} rsqrt(deg_s*deg_d) * (h_s @ W.T + b)

Device strategy (dst-sharded, SPMD over 8 cores, one instruction stream):
  - dst nodes are degree-sorted and grouped into windows of 512 slots; a
    window with max in-degree C is processed as C chunk-matmuls.
  - chunk = [128 in_feat, 512 slots] bf16 tile where column s holds the
    weighted source row w_e * h_src of dst slot s's c-th incoming edge
    (w_e = rs_src*rs_dst baked in; missing edges = zero columns).
  - PE: psum[outf, slot] += Wt.T @ chunk accumulates over chunks, so PSUM
    accumulation IS the edge scatter-add; W.T stays the stationary operand.
    A K=1 matmul adds the bias term b (x) wsumrs (wsumrs_d = rs_d*sum rs_s).
  - ACT copies PSUM->SBUF, DMA writes [outf, slot] tiles to DRAM; the host
    inverse-permutes slots back to node order.
  - windows are dealt to cores in rounds of 8 (sorted by C desc); each round
    uses the max C in the round as a shared template so all 8 cores run the
    same instruction stream on different data.
Host (numpy) prepares the edge shards: degrees, rs = deg**-0.5, edge ranks
within dst, and the per-core streamed chunk tensors (h rows scaled by edge
weight, laid out [128 feat, chunks*512] partition-major for thick DMA
descriptors).
"""

import sys

sys.path.insert(0, "/opt/trn_rl_repo")

import numpy as np

N_NODES = 50000
D = 128
N_CORES = 8
WIN = 512           # dst slots per window (= PSUM bank: 512 f32/partition)
G = 4               # chunks per streamed SBUF tile (4KB/partition descriptors)
BUFS = 20           # stream tile pool depth
PF = 18             # prefetch tiles beyond current window's need
TSPLIT = 23         # split dst nodes with degree > TSPLIT across slots

_COMPILED = {}


def _preprocess(h, W, b, edges):
    import ml_dtypes
    bf16 = np.dtype(ml_dtypes.bfloat16)

    h = np.asarray(h, dtype=np.float32)
    W = np.asarray(W, dtype=np.float32)
    b = np.asarray(b, dtype=np.float32)
    loops = np.arange(N_NODES, dtype=np.int64)
    src = np.concatenate([np.asarray(edges[0], dtype=np.int64), loops])
    dst = np.concatenate([np.asarray(edges[1], dtype=np.int64), loops])

    deg = np.bincount(dst, minlength=N_NODES)  # >=1 (self loops)
    rs = deg.astype(np.float64) ** -0.5
    # wsumrs[d] = rs_d * sum_{e into d} rs_src   (bias coefficient)
    wsumrs = (np.bincount(dst, weights=rs[src], minlength=N_NODES) * rs
              ).astype(np.float32)

    # split high-degree dst nodes across several virtual slots (flattens the
    # per-round chunk template; host sums the partial columns afterwards)
    kparts = -(-deg // TSPLIT)                          # parts per node
    vbase = np.zeros(N_NODES + 1, np.int64)
    vbase[1:] = np.cumsum(kparts)
    nv = int(vbase[-1])
    vnode = np.repeat(np.arange(N_NODES), kparts)       # virtual -> node
    part_idx = np.arange(nv) - vbase[vnode]
    vdeg = deg[vnode] // kparts[vnode] + (part_idx < deg[vnode] % kparts[vnode])

    # degree-sorted windows of WIN slots over virtual nodes
    order = np.argsort(vdeg, kind="stable")
    NW = N_CORES * (-(-nv // (WIN * N_CORES)))          # windows (padded)
    slots_total = NW * WIN
    assert nv <= slots_total
    slot_of = np.empty(nv, np.int64)
    slot_of[order] = np.arange(nv)
    degs_p = np.zeros(slots_total, np.int64)
    degs_p[:nv] = vdeg[order]
    C_w = np.maximum(degs_p.reshape(NW, WIN).max(axis=1), 1)

    # deal windows to cores in rounds of 8, sorted by C desc; shared template
    worder = np.argsort(-C_w, kind="stable")
    NR = NW // N_CORES
    win_round = np.empty(NW, np.int64)
    win_core = np.empty(NW, np.int64)
    win_round[worder] = np.arange(NW) // N_CORES
    win_core[worder] = np.arange(NW) % N_CORES
    C_template = C_w[worder].reshape(NR, N_CORES).max(axis=1)
    off = np.zeros(NR + 1, np.int64)
    off[1:] = np.cumsum(C_template)
    NCH = int(off[-1])                                  # chunks per core
    NCHp = -(-NCH // G) * G

    # per-edge placement: (core, chunk, slot) via virtual dst slots
    es = np.argsort(dst, kind="stable")
    starts = np.searchsorted(dst[es], np.arange(N_NODES))
    rank = np.empty(dst.size, np.int64)
    rank[es] = np.arange(dst.size) - starts[dst[es]]
    kd = kparts[dst]
    vdst = vbase[dst] + rank % kd                       # virtual dst node
    vrank = rank // kd                                  # rank within part
    gslot = slot_of[vdst]
    w_e = gslot // WIN
    s_e = gslot % WIN
    j_e = win_round[w_e]
    c_e = win_core[w_e]
    col = (off[j_e] + vrank) * WIN + s_e                # column in core stream
    wgt = (rs[src] * rs[dst]).astype(np.float32)

    # per-slot metadata rows (wsumrs on part 0 + output node mapping)
    g_all = np.arange(slots_total)
    w_all = g_all // WIN
    pos_all = win_round[w_all] * WIN + (g_all % WIN)
    core_all = win_core[w_all]
    node_all = np.full(slots_total, -1, np.int64)
    node_all[:nv] = vnode[order]
    ws_all = np.zeros(slots_total, np.float32)
    ws_all[:nv] = np.where(part_idx[order] == 0, wsumrs[vnode[order]], 0.0)
    wrow = np.zeros((N_CORES, NR * WIN), np.float32)
    node_at = np.full((N_CORES, NR * WIN), -1, np.int64)
    wrow[core_all, pos_all] = ws_all
    node_at[core_all, pos_all] = node_all

    Wt = np.ascontiguousarray(W.T).astype(bf16)
    brow = b.reshape(1, D).astype(bf16)

    in_maps = []
    for c in range(N_CORES):
        m = c_e == c
        vals = (h[src[m]] * wgt[m][:, None]).astype(bf16)       # [E_c, 128]
        sarr = np.zeros((NCHp * WIN, D), bf16)
        sarr[col[m]] = vals
        in_maps.append({
            "stream": np.ascontiguousarray(sarr.T),             # [128, cols]
            "wsum": np.ascontiguousarray(wrow[c:c + 1]).astype(bf16),
            "Wt": Wt, "b": brow,
        })

    geom = dict(C_template=tuple(int(x) for x in C_template), NCHp=NCHp)
    return in_maps, node_at, geom


def _build_nc(geom):
    import concourse.bacc as bacc
    import concourse.mybir as mybir
    import concourse.tile as tile

    Ct = geom["C_template"]
    NR = len(Ct)
    NCHp = geom["NCHp"]
    NT = NCHp // G
    bf16, f32 = mybir.dt.bfloat16, mybir.dt.float32

    nc = bacc.Bacc("TRN2", target_bir_lowering=False, debug=False,
                   num_devices=N_CORES)
    stream_d = nc.declare_dram_parameter("stream", [D, NCHp * WIN], bf16,
                                         isOutput=False)
    wsum_d = nc.declare_dram_parameter("wsum", [1, NR * WIN], bf16,
                                       isOutput=False)
    Wt_d = nc.declare_dram_parameter("Wt", [D, D], bf16, isOutput=False)
    b_d = nc.declare_dram_parameter("b", [1, D], bf16, isOutput=False)
    out_d = nc.declare_dram_parameter("out", [D, NR * WIN], bf16, isOutput=True)

    with tile.TileContext(nc) as tc:
        with (
            tc.tile_pool(name="const", bufs=1) as cpool,
            tc.tile_pool(name="xs", bufs=BUFS) as xs,
            tc.tile_pool(name="wp", bufs=3) as wp,
            tc.tile_pool(name="ps", bufs=4, space="PSUM") as ps,
        ):
            Wt_t = cpool.tile([D, D], bf16)
            nc.sync.dma_start(out=Wt_t[:], in_=Wt_d[:])
            b_t = cpool.tile([1, D], bf16)
            nc.sync.dma_start(out=b_t[:], in_=b_d[:])
            wsum_t = cpool.tile([1, NR * WIN], bf16)
            nc.sync.dma_start(out=wsum_t[:], in_=wsum_d[:])

            tiles = [None] * NT
            issued = 0
            issuers = (nc.sync, nc.scalar)

            def issue():
                nonlocal issued
                t = xs.tile([D, G * WIN], bf16, tag="x")
                issuers[issued % len(issuers)].dma_start(
                    out=t[:],
                    in_=stream_d[:, issued * G * WIN:(issued + 1) * G * WIN])
                tiles[issued] = t
                issued += 1

            off = 0
            osb = None
            for j in range(NR):
                need = off + Ct[j]
                want = min(NT, -(-need // G) + PF)
                while issued < want:
                    issue()
                pacc = ps.tile([D, WIN], f32, tag="acc")
                nc.tensor.matmul(out=pacc[:], lhsT=b_t[:],
                                 rhs=wsum_t[:, j * WIN:(j + 1) * WIN],
                                 start=True, stop=False)
                for t in range(Ct[j]):
                    c = off + t
                    xt = tiles[c // G][:, (c % G) * WIN:(c % G + 1) * WIN]
                    nc.tensor.matmul(out=pacc[:], lhsT=Wt_t[:], rhs=xt,
                                     start=False, stop=(t == Ct[j] - 1))
                off = need
                osb = wp.tile([D, WIN], bf16, tag="o")
                nc.scalar.copy(out=osb[:], in_=pacc[:])
                nc.sync.dma_start(out=out_d[:, j * WIN:(j + 1) * WIN],
                                  in_=osb[:])

    nc.finalize()
    return nc


def _get_nc(geom):
    key = (geom["C_template"], geom["NCHp"])
    if key not in _COMPILED:
        _COMPILED[key] = _build_nc(geom)
    return _COMPILED[key]


def _assemble(res, node_at):
    out = np.zeros((N_NODES, D), np.float32)
    for c in range(N_CORES):
        valid = node_at[c] >= 0
        np.add.at(out, node_at[c][valid],
                  res.results[c]["out"][:, valid].T.astype(np.float32))
    return out


def kernel(h, W, b, edges):
    from concourse.bass_utils import run_bass_kernel_spmd

    in_maps, node_at, geom = _preprocess(h, W, b, edges)
    nc = _get_nc(geom)
    res = None
    last_exc = None
    for _attempt in range(3):
        try:
            res = run_bass_kernel_spmd(nc, in_maps, list(range(N_CORES)))
            break
        except Exception as e:  # transient axon/NRT hiccups
            last_exc = e
            import time
            time.sleep(2.0)
    if res is None:
        raise last_exc
    return _assemble(res, node_at)


# revision 24
# speedup vs baseline: 1.0242x; 1.0242x over previous
"""GCN layer (symmetric-normalized, self-loops) on 8 Trainium2 NeuronCores.

out[d] = sum_{e:(s,d)} rsqrt(deg_s*deg_d) * (h_s @ W.T + b)

Device strategy (dst-sharded, SPMD over 8 cores, one instruction stream):
  - dst nodes (degree > TSPLIT ones split across virtual slots, host sums
    the partials) are degree-sorted into windows of 512 slots; a window
    with max in-degree C is processed as C chunk-matmuls.
  - chunk = [128 in_feat, 512 slots] bf16 tile where column s holds the
    weighted source row w_e * h_src of dst slot s's c-th incoming edge
    (w_e = rs_src*rs_dst baked in; missing edges = zero columns).
  - PE: psum[outf, slot] += Wt.T @ chunk accumulates over chunks, so PSUM
    accumulation IS the edge scatter-add; W.T stays the stationary operand.
    A K=1 matmul adds the bias term b (x) wsumrs (wsumrs_d = rs_d*sum rs_s).
  - ACT copies PSUM->SBUF (bf16), DMA writes [outf, slot] tiles to DRAM;
    the host inverse-permutes slots back to node order.
  - windows are dealt to cores in rounds of 8 (sorted by C desc); each round
    uses the max C in the round as a shared template so all 8 cores run the
    same instruction stream on different data.
  - no gather anywhere: the only DMA is the sequential chunk stream
    ([128, chunks*512] partition-major, 4KB descriptors, issued round-robin
    from the SP and ACT sequencers) - the kernel runs at the chip HBM
    roofline (~74us of DMA busy for ~28MB/core).
Host (numpy) prepares the edge shards: degrees, rs = deg**-0.5, edge ranks
within dst, and the per-core streamed chunk tensors (h rows scaled by edge
weight).
"""

import sys

sys.path.insert(0, "/opt/trn_rl_repo")

import numpy as np

N_NODES = 50000
D = 128
N_CORES = 8
WIN = 512           # dst slots per window (= PSUM bank: 512 f32/partition)
G = 4               # chunks per streamed SBUF tile (4KB/partition descriptors)
BUFS = 20           # stream tile pool depth
PF = 18             # prefetch tiles beyond current window's need
TSPLIT = 23         # split dst nodes with degree > TSPLIT across slots

_COMPILED = {}


def _preprocess(h, W, b, edges):
    import ml_dtypes
    bf16 = np.dtype(ml_dtypes.bfloat16)

    h = np.asarray(h, dtype=np.float32)
    W = np.asarray(W, dtype=np.float32)
    b = np.asarray(b, dtype=np.float32)
    loops = np.arange(N_NODES, dtype=np.int64)
    src = np.concatenate([np.asarray(edges[0], dtype=np.int64), loops])
    dst = np.concatenate([np.asarray(edges[1], dtype=np.int64), loops])

    deg = np.bincount(dst, minlength=N_NODES)  # >=1 (self loops)
    rs = deg.astype(np.float64) ** -0.5
    # wsumrs[d] = rs_d * sum_{e into d} rs_src   (bias coefficient)
    wsumrs = (np.bincount(dst, weights=rs[src], minlength=N_NODES) * rs
              ).astype(np.float32)

    # split high-degree dst nodes across several virtual slots (flattens the
    # per-round chunk template; host sums the partial columns afterwards)
    kparts = -(-deg // TSPLIT)                          # parts per node
    vbase = np.zeros(N_NODES + 1, np.int64)
    vbase[1:] = np.cumsum(kparts)
    nv = int(vbase[-1])
    vnode = np.repeat(np.arange(N_NODES), kparts)       # virtual -> node
    part_idx = np.arange(nv) - vbase[vnode]
    vdeg = deg[vnode] // kparts[vnode] + (part_idx < deg[vnode] % kparts[vnode])

    # degree-sorted windows of WIN slots over virtual nodes
    order = np.argsort(vdeg, kind="stable")
    NW = N_CORES * (-(-nv // (WIN * N_CORES)))          # windows (padded)
    slots_total = NW * WIN
    assert nv <= slots_total
    slot_of = np.empty(nv, np.int64)
    slot_of[order] = np.arange(nv)
    degs_p = np.zeros(slots_total, np.int64)
    degs_p[:nv] = vdeg[order]
    C_w = np.maximum(degs_p.reshape(NW, WIN).max(axis=1), 1)

    # deal windows to cores in rounds of 8, sorted by C desc; shared template
    worder = np.argsort(-C_w, kind="stable")
    NR = NW // N_CORES
    win_round = np.empty(NW, np.int64)
    win_core = np.empty(NW, np.int64)
    win_round[worder] = np.arange(NW) // N_CORES
    win_core[worder] = np.arange(NW) % N_CORES
    C_template = C_w[worder].reshape(NR, N_CORES).max(axis=1)
    off = np.zeros(NR + 1, np.int64)
    off[1:] = np.cumsum(C_template)
    NCH = int(off[-1])                                  # chunks per core
    NCHp = -(-NCH // G) * G

    # per-edge placement: (core, chunk, slot) via virtual dst slots
    es = np.argsort(dst, kind="stable")
    starts = np.searchsorted(dst[es], np.arange(N_NODES))
    rank = np.empty(dst.size, np.int64)
    rank[es] = np.arange(dst.size) - starts[dst[es]]
    kd = kparts[dst]
    vdst = vbase[dst] + rank % kd                       # virtual dst node
    vrank = rank // kd                                  # rank within part
    gslot = slot_of[vdst]
    w_e = gslot // WIN
    s_e = gslot % WIN
    j_e = win_round[w_e]
    c_e = win_core[w_e]
    col = (off[j_e] + vrank) * WIN + s_e                # column in core stream
    wgt = (rs[src] * rs[dst]).astype(np.float32)

    # per-slot metadata rows (wsumrs on part 0 + output node mapping)
    g_all = np.arange(slots_total)
    w_all = g_all // WIN
    pos_all = win_round[w_all] * WIN + (g_all % WIN)
    core_all = win_core[w_all]
    node_all = np.full(slots_total, -1, np.int64)
    node_all[:nv] = vnode[order]
    ws_all = np.zeros(slots_total, np.float32)
    ws_all[:nv] = np.where(part_idx[order] == 0, wsumrs[vnode[order]], 0.0)
    wrow = np.zeros((N_CORES, NR * WIN), np.float32)
    node_at = np.full((N_CORES, NR * WIN), -1, np.int64)
    wrow[core_all, pos_all] = ws_all
    node_at[core_all, pos_all] = node_all

    Wt = np.ascontiguousarray(W.T).astype(bf16)
    brow = b.reshape(1, D).astype(bf16)

    in_maps = []
    for c in range(N_CORES):
        m = c_e == c
        vals = (h[src[m]] * wgt[m][:, None]).astype(bf16)       # [E_c, 128]
        sarr = np.zeros((NCHp * WIN, D), bf16)
        sarr[col[m]] = vals
        in_maps.append({
            "stream": np.ascontiguousarray(sarr.T),             # [128, cols]
            "wsum": np.ascontiguousarray(wrow[c:c + 1]).astype(bf16),
            "Wt": Wt, "b": brow,
        })

    geom = dict(C_template=tuple(int(x) for x in C_template), NCHp=NCHp)
    return in_maps, node_at, geom


def _build_nc(geom):
    import concourse.bacc as bacc
    import concourse.mybir as mybir
    import concourse.tile as tile

    Ct = geom["C_template"]
    NR = len(Ct)
    NCHp = geom["NCHp"]
    NT = NCHp // G
    bf16, f32 = mybir.dt.bfloat16, mybir.dt.float32

    nc = bacc.Bacc("TRN2", target_bir_lowering=False, debug=False,
                   num_devices=N_CORES)
    stream_d = nc.declare_dram_parameter("stream", [D, NCHp * WIN], bf16,
                                         isOutput=False)
    wsum_d = nc.declare_dram_parameter("wsum", [1, NR * WIN], bf16,
                                       isOutput=False)
    Wt_d = nc.declare_dram_parameter("Wt", [D, D], bf16, isOutput=False)
    b_d = nc.declare_dram_parameter("b", [1, D], bf16, isOutput=False)
    out_d = nc.declare_dram_parameter("out", [D, NR * WIN], bf16, isOutput=True)

    with tile.TileContext(nc) as tc:
        with (
            tc.tile_pool(name="const", bufs=1) as cpool,
            tc.tile_pool(name="xs", bufs=BUFS) as xs,
            tc.tile_pool(name="wp", bufs=3) as wp,
            tc.tile_pool(name="ps", bufs=4, space="PSUM") as ps,
        ):
            Wt_t = cpool.tile([D, D], bf16)
            nc.sync.dma_start(out=Wt_t[:], in_=Wt_d[:])
            b_t = cpool.tile([1, D], bf16)
            nc.sync.dma_start(out=b_t[:], in_=b_d[:])
            wsum_t = cpool.tile([1, NR * WIN], bf16)
            nc.sync.dma_start(out=wsum_t[:], in_=wsum_d[:])

            tiles = [None] * NT
            issued = 0
            issuers = (nc.sync, nc.scalar)

            def issue():
                nonlocal issued
                t = xs.tile([D, G * WIN], bf16, tag="x")
                issuers[issued % len(issuers)].dma_start(
                    out=t[:],
                    in_=stream_d[:, issued * G * WIN:(issued + 1) * G * WIN])
                tiles[issued] = t
                issued += 1

            off = 0
            osb = None
            for j in range(NR):
                need = off + Ct[j]
                want = min(NT, -(-need // G) + PF)
                while issued < want:
                    issue()
                pacc = ps.tile([D, WIN], f32, tag="acc")
                nc.tensor.matmul(out=pacc[:], lhsT=b_t[:],
                                 rhs=wsum_t[:, j * WIN:(j + 1) * WIN],
                                 start=True, stop=False)
                for t in range(Ct[j]):
                    c = off + t
                    xt = tiles[c // G][:, (c % G) * WIN:(c % G + 1) * WIN]
                    nc.tensor.matmul(out=pacc[:], lhsT=Wt_t[:], rhs=xt,
                                     start=False, stop=(t == Ct[j] - 1))
                off = need
                osb = wp.tile([D, WIN], bf16, tag="o")
                nc.scalar.copy(out=osb[:], in_=pacc[:])
                nc.sync.dma_start(out=out_d[:, j * WIN:(j + 1) * WIN],
                                  in_=osb[:])

    nc.finalize()
    return nc


def _get_nc(geom):
    key = (geom["C_template"], geom["NCHp"])
    if key not in _COMPILED:
        _COMPILED[key] = _build_nc(geom)
    return _COMPILED[key]


def _assemble(res, node_at):
    out = np.zeros((N_NODES, D), np.float32)
    for c in range(N_CORES):
        valid = node_at[c] >= 0
        np.add.at(out, node_at[c][valid],
                  res.results[c]["out"][:, valid].T.astype(np.float32))
    return out


def kernel(h, W, b, edges):
    from concourse.bass_utils import run_bass_kernel_spmd

    in_maps, node_at, geom = _preprocess(h, W, b, edges)
    nc = _get_nc(geom)
    res = None
    last_exc = None
    for _attempt in range(3):
        try:
            res = run_bass_kernel_spmd(nc, in_maps, list(range(N_CORES)))
            break
        except Exception as e:  # transient axon/NRT hiccups
            last_exc = e
            import time
            time.sleep(2.0)
    if res is None:
        raise last_exc
    return _assemble(res, node_at)


# revision 27
# speedup vs baseline: 1.0261x; 1.0018x over previous
"""GCN layer (symmetric-normalized, self-loops) on 8 Trainium2 NeuronCores.

out[d] = sum_{e:(s,d)} rsqrt(deg_s*deg_d) * (h_s @ W.T + b)

Device strategy (dst-sharded, SPMD over 8 cores, one instruction stream):
  - dst nodes (degree > TSPLIT ones split across virtual slots, host sums
    the partials) are degree-sorted into windows of 512 slots; a window
    with max in-degree C is processed as C chunk-matmuls.
  - chunk = [128 in_feat, 512 slots] bf16 tile where column s holds the
    weighted source row w_e * h_src of dst slot s's c-th incoming edge
    (w_e = rs_src*rs_dst baked in; missing edges = zero columns).
  - PE: psum[outf, slot] += Wt.T @ chunk accumulates over chunks, so PSUM
    accumulation IS the edge scatter-add; W.T stays the stationary operand.
    A K=1 matmul adds the bias term b (x) wsumrs (wsumrs_d = rs_d*sum rs_s).
  - ACT copies PSUM->SBUF (bf16), DMA writes [outf, slot] tiles to DRAM;
    the host inverse-permutes slots back to node order.
  - windows are dealt to cores in rounds of 8 (sorted by C desc); each round
    uses the max C in the round as a shared template so all 8 cores run the
    same instruction stream on different data.
  - no gather anywhere: the only DMA is the sequential chunk stream
    ([128, chunks*512] partition-major, 4KB descriptors, issued round-robin
    from the SP and ACT sequencers) - the kernel runs at the chip HBM
    roofline (~74us of DMA busy for ~28MB/core).
Host (numpy) prepares the edge shards: degrees, rs = deg**-0.5, edge ranks
within dst, and the per-core streamed chunk tensors (h rows scaled by edge
weight).
"""

import sys

sys.path.insert(0, "/opt/trn_rl_repo")

import numpy as np

N_NODES = 50000
D = 128
N_CORES = 8
WIN = 512           # dst slots per window (= PSUM bank: 512 f32/partition)
G = 4               # chunks per streamed SBUF tile (4KB/partition descriptors)
BUFS = 20           # stream tile pool depth
PF = 18             # prefetch tiles beyond current window's need
TSPLIT = 23         # split dst nodes with degree > TSPLIT across slots

_COMPILED = {}


def _preprocess(h, W, b, edges):
    import ml_dtypes
    bf16 = np.dtype(ml_dtypes.bfloat16)

    h = np.asarray(h, dtype=np.float32)
    W = np.asarray(W, dtype=np.float32)
    b = np.asarray(b, dtype=np.float32)
    loops = np.arange(N_NODES, dtype=np.int64)
    src = np.concatenate([np.asarray(edges[0], dtype=np.int64), loops])
    dst = np.concatenate([np.asarray(edges[1], dtype=np.int64), loops])

    deg = np.bincount(dst, minlength=N_NODES)  # >=1 (self loops)
    rs = deg.astype(np.float64) ** -0.5
    # wsumrs[d] = rs_d * sum_{e into d} rs_src   (bias coefficient)
    wsumrs = (np.bincount(dst, weights=rs[src], minlength=N_NODES) * rs
              ).astype(np.float32)

    # split high-degree dst nodes across several virtual slots (flattens the
    # per-round chunk template; host sums the partial columns afterwards)
    kparts = -(-deg // TSPLIT)                          # parts per node
    vbase = np.zeros(N_NODES + 1, np.int64)
    vbase[1:] = np.cumsum(kparts)
    nv = int(vbase[-1])
    vnode = np.repeat(np.arange(N_NODES), kparts)       # virtual -> node
    part_idx = np.arange(nv) - vbase[vnode]
    vdeg = deg[vnode] // kparts[vnode] + (part_idx < deg[vnode] % kparts[vnode])

    # degree-sorted windows of WIN slots over virtual nodes
    order = np.argsort(vdeg, kind="stable")
    NW = N_CORES * (-(-nv // (WIN * N_CORES)))          # windows (padded)
    slots_total = NW * WIN
    assert nv <= slots_total
    slot_of = np.empty(nv, np.int64)
    slot_of[order] = np.arange(nv)
    degs_p = np.zeros(slots_total, np.int64)
    degs_p[:nv] = vdeg[order]
    C_w = np.maximum(degs_p.reshape(NW, WIN).max(axis=1), 1)

    # deal windows to cores in rounds of 8, sorted by C desc; shared template
    worder = np.argsort(-C_w, kind="stable")
    NR = NW // N_CORES
    win_round = np.empty(NW, np.int64)
    win_core = np.empty(NW, np.int64)
    win_round[worder] = np.arange(NW) // N_CORES
    win_core[worder] = np.arange(NW) % N_CORES
    C_template = C_w[worder].reshape(NR, N_CORES).max(axis=1)
    off = np.zeros(NR + 1, np.int64)
    off[1:] = np.cumsum(C_template)
    NCH = int(off[-1])                                  # chunks per core
    NCHp = -(-NCH // G) * G

    # per-edge placement: (core, chunk, slot) via virtual dst slots
    es = np.argsort(dst, kind="stable")
    starts = np.searchsorted(dst[es], np.arange(N_NODES))
    rank = np.empty(dst.size, np.int64)
    rank[es] = np.arange(dst.size) - starts[dst[es]]
    kd = kparts[dst]
    vdst = vbase[dst] + rank % kd                       # virtual dst node
    vrank = rank // kd                                  # rank within part
    gslot = slot_of[vdst]
    w_e = gslot // WIN
    s_e = gslot % WIN
    j_e = win_round[w_e]
    c_e = win_core[w_e]
    col = (off[j_e] + vrank) * WIN + s_e                # column in core stream
    wgt = (rs[src] * rs[dst]).astype(np.float32)

    # per-slot metadata rows (wsumrs on part 0 + output node mapping)
    g_all = np.arange(slots_total)
    w_all = g_all // WIN
    pos_all = win_round[w_all] * WIN + (g_all % WIN)
    core_all = win_core[w_all]
    node_all = np.full(slots_total, -1, np.int64)
    node_all[:nv] = vnode[order]
    ws_all = np.zeros(slots_total, np.float32)
    ws_all[:nv] = np.where(part_idx[order] == 0, wsumrs[vnode[order]], 0.0)
    wrow = np.zeros((N_CORES, NR * WIN), np.float32)
    node_at = np.full((N_CORES, NR * WIN), -1, np.int64)
    wrow[core_all, pos_all] = ws_all
    node_at[core_all, pos_all] = node_all

    Wt = np.ascontiguousarray(W.T).astype(bf16)
    brow = b.reshape(1, D).astype(bf16)

    in_maps = []
    for c in range(N_CORES):
        m = c_e == c
        vals = (h[src[m]] * wgt[m][:, None]).astype(bf16)       # [E_c, 128]
        sarr = np.zeros((NCHp * WIN, D), bf16)
        sarr[col[m]] = vals
        in_maps.append({
            "stream": np.ascontiguousarray(sarr.T),             # [128, cols]
            "wsum": np.ascontiguousarray(wrow[c:c + 1]).astype(bf16),
            "Wt": Wt, "b": brow,
        })

    geom = dict(C_template=tuple(int(x) for x in C_template), NCHp=NCHp)
    return in_maps, node_at, geom


def _build_nc(geom):
    import concourse.bacc as bacc
    import concourse.mybir as mybir
    import concourse.tile as tile

    Ct = geom["C_template"]
    NR = len(Ct)
    NCHp = geom["NCHp"]
    NT = NCHp // G
    bf16, f32 = mybir.dt.bfloat16, mybir.dt.float32

    nc = bacc.Bacc("TRN2", target_bir_lowering=False, debug=False,
                   num_devices=N_CORES)
    stream_d = nc.declare_dram_parameter("stream", [D, NCHp * WIN], bf16,
                                         isOutput=False)
    wsum_d = nc.declare_dram_parameter("wsum", [1, NR * WIN], bf16,
                                       isOutput=False)
    Wt_d = nc.declare_dram_parameter("Wt", [D, D], bf16, isOutput=False)
    b_d = nc.declare_dram_parameter("b", [1, D], bf16, isOutput=False)
    out_d = nc.declare_dram_parameter("out", [D, NR * WIN], bf16, isOutput=True)

    with tile.TileContext(nc) as tc:
        with (
            tc.tile_pool(name="const", bufs=1) as cpool,
            tc.tile_pool(name="xs", bufs=BUFS) as xs,
            tc.tile_pool(name="wp", bufs=3) as wp,
            tc.tile_pool(name="ps", bufs=4, space="PSUM") as ps,
        ):
            Wt_t = cpool.tile([D, D], bf16)
            nc.sync.dma_start(out=Wt_t[:], in_=Wt_d[:])
            b_t = cpool.tile([1, D], bf16)
            nc.sync.dma_start(out=b_t[:], in_=b_d[:])
            wsum_t = cpool.tile([1, NR * WIN], bf16)
            nc.sync.dma_start(out=wsum_t[:], in_=wsum_d[:])

            tiles = [None] * NT
            issued = 0
            issuers = (nc.sync, nc.scalar)

            def issue():
                nonlocal issued
                t = xs.tile([D, G * WIN], bf16, tag="x")
                issuers[issued % len(issuers)].dma_start(
                    out=t[:],
                    in_=stream_d[:, issued * G * WIN:(issued + 1) * G * WIN])
                tiles[issued] = t
                issued += 1

            off = 0
            osb = None
            for j in range(NR):
                need = off + Ct[j]
                want = min(NT, -(-need // G) + PF)
                while issued < want:
                    issue()
                pacc = ps.tile([D, WIN], f32, tag="acc")
                nc.tensor.matmul(out=pacc[:], lhsT=b_t[:],
                                 rhs=wsum_t[:, j * WIN:(j + 1) * WIN],
                                 start=True, stop=False)
                for t in range(Ct[j]):
                    c = off + t
                    xt = tiles[c // G][:, (c % G) * WIN:(c % G + 1) * WIN]
                    nc.tensor.matmul(out=pacc[:], lhsT=Wt_t[:], rhs=xt,
                                     start=False, stop=(t == Ct[j] - 1))
                off = need
                osb = wp.tile([D, WIN], bf16, tag="o")
                nc.scalar.copy(out=osb[:], in_=pacc[:])
                nc.sync.dma_start(out=out_d[:, j * WIN:(j + 1) * WIN],
                                  in_=osb[:])

    nc.finalize()
    return nc


def _get_nc(geom):
    key = (geom["C_template"], geom["NCHp"])
    if key not in _COMPILED:
        _COMPILED[key] = _build_nc(geom)
    return _COMPILED[key]


def _assemble(res, node_at):
    out = np.zeros((N_NODES, D), np.float32)
    for c in range(N_CORES):
        valid = node_at[c] >= 0
        np.add.at(out, node_at[c][valid],
                  res.results[c]["out"][:, valid].T.astype(np.float32))
    return out


def kernel(h, W, b, edges):
    from concourse.bass_utils import run_bass_kernel_spmd

    in_maps, node_at, geom = _preprocess(h, W, b, edges)
    nc = _get_nc(geom)
    res = None
    last_exc = None
    for _attempt in range(3):
        try:
            res = run_bass_kernel_spmd(nc, in_maps, list(range(N_CORES)))
            break
        except Exception as e:  # transient axon/NRT hiccups
            last_exc = e
            import time
            time.sleep(2.0)
    if res is None:
        raise last_exc
    return _assemble(res, node_at)


# revision 29
# speedup vs baseline: 1.0393x; 1.0129x over previous
"""GCN layer (symmetric-normalized, self-loops) on 8 Trainium2 NeuronCores.

out[d] = sum_{e:(s,d)} rsqrt(deg_s*deg_d) * (h_s @ W.T + b)

Device strategy (dst-sharded, SPMD over 8 cores, one instruction stream):
  - dst nodes (degree > TSPLIT ones split across virtual slots, host sums
    the partials) are degree-sorted into windows of 512 slots; a window
    with max in-degree C is processed as C chunk-matmuls.
  - chunk = [128 in_feat, 512 slots] bf16 tile where column s holds the
    weighted source row w_e * h_src of dst slot s's c-th incoming edge
    (w_e = rs_src*rs_dst baked in; missing edges = zero columns).
  - PE: psum[outf, slot] += Wt.T @ chunk accumulates over chunks, so PSUM
    accumulation IS the edge scatter-add; W.T stays the stationary operand.
    A K=1 matmul adds the bias term b (x) wsumrs (wsumrs_d = rs_d*sum rs_s).
  - ACT copies PSUM->SBUF (bf16), DMA writes [outf, slot] tiles to DRAM;
    the host inverse-permutes slots back to node order.
  - windows are dealt to cores in rounds of 8 (sorted by C desc); each round
    uses the max C in the round as a shared template so all 8 cores run the
    same instruction stream on different data.
  - no gather anywhere: the only DMA is the sequential chunk stream
    ([128, chunks*512] partition-major, 4KB descriptors, issued round-robin
    from the SP and ACT sequencers) - the kernel runs at the chip HBM
    roofline (~74us of DMA busy for ~28MB/core).
Host (numpy) prepares the edge shards: degrees, rs = deg**-0.5, edge ranks
within dst, and the per-core streamed chunk tensors (h rows scaled by edge
weight).
"""

import sys

sys.path.insert(0, "/opt/trn_rl_repo")

import numpy as np

N_NODES = 50000
D = 128
N_CORES = 8
WIN = 512           # dst slots per window (= PSUM bank: 512 f32/partition)
G = 4               # chunks per streamed SBUF tile (4KB/partition descriptors)
BUFS = 20           # stream tile pool depth
PF = 18             # prefetch tiles beyond current window's need
TSPLIT = 23         # split dst nodes with degree > TSPLIT across slots

import os
_PAIR_OUT = os.environ.get("KPAIR", "0") == "1"

_COMPILED = {}


def _preprocess(h, W, b, edges):
    import ml_dtypes
    bf16 = np.dtype(ml_dtypes.bfloat16)

    h = np.asarray(h, dtype=np.float32)
    W = np.asarray(W, dtype=np.float32)
    b = np.asarray(b, dtype=np.float32)
    loops = np.arange(N_NODES, dtype=np.int64)
    src = np.concatenate([np.asarray(edges[0], dtype=np.int64), loops])
    dst = np.concatenate([np.asarray(edges[1], dtype=np.int64), loops])

    deg = np.bincount(dst, minlength=N_NODES)  # >=1 (self loops)
    rs = deg.astype(np.float64) ** -0.5
    # wsumrs[d] = rs_d * sum_{e into d} rs_src   (bias coefficient)
    wsumrs = (np.bincount(dst, weights=rs[src], minlength=N_NODES) * rs
              ).astype(np.float32)

    # split high-degree dst nodes across several virtual slots (flattens the
    # per-round chunk template; host sums the partial columns afterwards)
    kparts = -(-deg // TSPLIT)                          # parts per node
    vbase = np.zeros(N_NODES + 1, np.int64)
    vbase[1:] = np.cumsum(kparts)
    nv = int(vbase[-1])
    vnode = np.repeat(np.arange(N_NODES), kparts)       # virtual -> node
    part_idx = np.arange(nv) - vbase[vnode]
    vdeg = deg[vnode] // kparts[vnode] + (part_idx < deg[vnode] % kparts[vnode])

    # degree-sorted windows of WIN slots over virtual nodes
    order = np.argsort(vdeg, kind="stable")
    NW = N_CORES * (-(-nv // (WIN * N_CORES)))          # windows (padded)
    slots_total = NW * WIN
    assert nv <= slots_total
    slot_of = np.empty(nv, np.int64)
    slot_of[order] = np.arange(nv)
    degs_p = np.zeros(slots_total, np.int64)
    degs_p[:nv] = vdeg[order]
    C_w = np.maximum(degs_p.reshape(NW, WIN).max(axis=1), 1)

    # deal windows to cores in rounds of 8, sorted by C desc; shared template
    worder = np.argsort(-C_w, kind="stable")
    NR = NW // N_CORES
    win_round = np.empty(NW, np.int64)
    win_core = np.empty(NW, np.int64)
    win_round[worder] = np.arange(NW) // N_CORES
    win_core[worder] = np.arange(NW) % N_CORES
    C_template = C_w[worder].reshape(NR, N_CORES).max(axis=1)
    off = np.zeros(NR + 1, np.int64)
    off[1:] = np.cumsum(C_template)
    NCH = int(off[-1])                                  # chunks per core
    NCHp = -(-NCH // G) * G

    # per-edge placement: (core, chunk, slot) via virtual dst slots
    es = np.argsort(dst, kind="stable")
    starts = np.searchsorted(dst[es], np.arange(N_NODES))
    rank = np.empty(dst.size, np.int64)
    rank[es] = np.arange(dst.size) - starts[dst[es]]
    kd = kparts[dst]
    vdst = vbase[dst] + rank % kd                       # virtual dst node
    vrank = rank // kd                                  # rank within part
    gslot = slot_of[vdst]
    w_e = gslot // WIN
    s_e = gslot % WIN
    j_e = win_round[w_e]
    c_e = win_core[w_e]
    col = (off[j_e] + vrank) * WIN + s_e                # column in core stream
    wgt = (rs[src] * rs[dst]).astype(np.float32)

    # per-slot metadata rows (wsumrs on part 0 + output node mapping)
    g_all = np.arange(slots_total)
    w_all = g_all // WIN
    pos_all = win_round[w_all] * WIN + (g_all % WIN)
    core_all = win_core[w_all]
    node_all = np.full(slots_total, -1, np.int64)
    node_all[:nv] = vnode[order]
    ws_all = np.zeros(slots_total, np.float32)
    ws_all[:nv] = np.where(part_idx[order] == 0, wsumrs[vnode[order]], 0.0)
    wrow = np.zeros((N_CORES, NR * WIN), np.float32)
    node_at = np.full((N_CORES, NR * WIN), -1, np.int64)
    wrow[core_all, pos_all] = ws_all
    node_at[core_all, pos_all] = node_all

    Wt = np.ascontiguousarray(W.T).astype(bf16)
    brow = b.reshape(1, D).astype(bf16)

    in_maps = []
    for c in range(N_CORES):
        m = c_e == c
        vals = (h[src[m]] * wgt[m][:, None]).astype(bf16)       # [E_c, 128]
        sarr = np.zeros((NCHp * WIN, D), bf16)
        sarr[col[m]] = vals
        in_maps.append({
            "stream": np.ascontiguousarray(sarr.T),             # [128, cols]
            "wsum": np.ascontiguousarray(wrow[c:c + 1]).astype(bf16),
            "Wt": Wt, "b": brow,
        })

    geom = dict(C_template=tuple(int(x) for x in C_template), NCHp=NCHp)
    return in_maps, node_at, geom


def _build_nc(geom):
    import concourse.bacc as bacc
    import concourse.mybir as mybir
    import concourse.tile as tile

    Ct = geom["C_template"]
    NR = len(Ct)
    NCHp = geom["NCHp"]
    NT = NCHp // G
    bf16, f32 = mybir.dt.bfloat16, mybir.dt.float32

    nc = bacc.Bacc("TRN2", target_bir_lowering=False, debug=False,
                   num_devices=N_CORES)
    stream_d = nc.declare_dram_parameter("stream", [D, NCHp * WIN], bf16,
                                         isOutput=False)
    wsum_d = nc.declare_dram_parameter("wsum", [1, NR * WIN], bf16,
                                       isOutput=False)
    Wt_d = nc.declare_dram_parameter("Wt", [D, D], bf16, isOutput=False)
    b_d = nc.declare_dram_parameter("b", [1, D], bf16, isOutput=False)
    out_d = nc.declare_dram_parameter("out", [D, NR * WIN], bf16, isOutput=True)

    with tile.TileContext(nc) as tc:
        with (
            tc.tile_pool(name="const", bufs=1) as cpool,
            tc.tile_pool(name="xs", bufs=BUFS) as xs,
            tc.tile_pool(name="wp", bufs=3) as wp,
            tc.tile_pool(name="ps", bufs=4, space="PSUM") as ps,
        ):
            Wt_t = cpool.tile([D, D], bf16)
            nc.sync.dma_start(out=Wt_t[:], in_=Wt_d[:])
            b_t = cpool.tile([1, D], bf16)
            nc.sync.dma_start(out=b_t[:], in_=b_d[:])
            wsum_t = cpool.tile([1, NR * WIN], bf16)
            nc.sync.dma_start(out=wsum_t[:], in_=wsum_d[:])

            tiles = [None] * NT
            issued = 0
            issuers = (nc.sync, nc.scalar)

            def issue():
                nonlocal issued
                t = xs.tile([D, G * WIN], bf16, tag="x")
                issuers[issued % len(issuers)].dma_start(
                    out=t[:],
                    in_=stream_d[:, issued * G * WIN:(issued + 1) * G * WIN])
                tiles[issued] = t
                issued += 1

            off = 0
            osb = None
            for j in range(NR):
                need = off + Ct[j]
                want = min(NT, -(-need // G) + PF)
                while issued < want:
                    issue()
                pacc = ps.tile([D, WIN], f32, tag="acc")
                nc.tensor.matmul(out=pacc[:], lhsT=b_t[:],
                                 rhs=wsum_t[:, j * WIN:(j + 1) * WIN],
                                 start=True, stop=False)
                for t in range(Ct[j]):
                    c = off + t
                    xt = tiles[c // G][:, (c % G) * WIN:(c % G + 1) * WIN]
                    nc.tensor.matmul(out=pacc[:], lhsT=Wt_t[:], rhs=xt,
                                     start=False, stop=(t == Ct[j] - 1))
                off = need
                if _PAIR_OUT:
                    # pair windows into one output tile -> 2KB descriptors,
                    # DMA'd from the scalar sequencer
                    if j % 2 == 0:
                        osb = wp.tile([D, 2 * WIN], bf16, tag="o")
                    half = (j % 2) * WIN
                    nc.scalar.copy(out=osb[:, half:half + WIN], in_=pacc[:])
                    if j % 2 == 1 or j == NR - 1:
                        j0 = (j // 2) * 2
                        w = (j - j0 + 1) * WIN
                        nc.scalar.dma_start(
                            out=out_d[:, j0 * WIN:j0 * WIN + w],
                            in_=osb[:, :w])
                else:
                    osb = wp.tile([D, WIN], bf16, tag="o")
                    nc.scalar.copy(out=osb[:], in_=pacc[:])
                    nc.sync.dma_start(out=out_d[:, j * WIN:(j + 1) * WIN],
                                      in_=osb[:])

    nc.finalize()
    return nc


def _get_nc(geom):
    key = (geom["C_template"], geom["NCHp"])
    if key not in _COMPILED:
        _COMPILED[key] = _build_nc(geom)
    return _COMPILED[key]


def _assemble(res, node_at):
    out = np.zeros((N_NODES, D), np.float32)
    for c in range(N_CORES):
        valid = node_at[c] >= 0
        np.add.at(out, node_at[c][valid],
                  res.results[c]["out"][:, valid].T.astype(np.float32))
    return out


def kernel(h, W, b, edges):
    from concourse.bass_utils import run_bass_kernel_spmd

    in_maps, node_at, geom = _preprocess(h, W, b, edges)
    nc = _get_nc(geom)
    res = None
    last_exc = None
    for _attempt in range(3):
        try:
            res = run_bass_kernel_spmd(nc, in_maps, list(range(N_CORES)))
            break
        except Exception as e:  # transient axon/NRT hiccups
            last_exc = e
            import time
            time.sleep(2.0)
    if res is None:
        raise last_exc
    return _assemble(res, node_at)


# revision 31
# speedup vs baseline: 1.1825x; 1.1378x over previous
"""GCN layer (symmetric-normalized, self-loops) on 8 Trainium2 NeuronCores.

out[d] = sum_{e:(s,d)} rsqrt(deg_s*deg_d) * (h_s @ W.T + b)

Device strategy (dst-sharded, SPMD over 8 cores, one instruction stream):
  - dst nodes (degree > TSPLIT ones split across virtual slots, host sums
    the partials) are degree-sorted into windows of 512 slots; a window
    with max in-degree C is processed as C chunk-matmuls.
  - chunk = [128 in_feat, 512 slots] bf16 tile where column s holds the
    weighted source row w_e * h_src of dst slot s's c-th incoming edge
    (w_e = rs_src*rs_dst baked in; missing edges = zero columns).
  - PE: psum[outf, slot] += Wt.T @ chunk accumulates over chunks, so PSUM
    accumulation IS the edge scatter-add; W.T stays the stationary operand.
    A K=1 matmul adds the bias term b (x) wsumrs (wsumrs_d = rs_d*sum rs_s).
  - ACT copies PSUM->SBUF (bf16), DMA writes [outf, slot] tiles to DRAM;
    the host inverse-permutes slots back to node order.
  - windows are dealt to cores in rounds of 8 (sorted by C desc); each round
    uses the max C in the round as a shared template so all 8 cores run the
    same instruction stream on different data.
  - no gather anywhere: the only DMA is the sequential chunk stream
    ([128, chunks*512] partition-major, 4KB descriptors, issued round-robin
    from the SP and ACT sequencers) - the kernel runs at the chip HBM
    roofline (~74us of DMA busy for ~28MB/core).
Host (numpy) prepares the edge shards: degrees, rs = deg**-0.5, edge ranks
within dst, and the per-core streamed chunk tensors (h rows scaled by edge
weight).
"""

import sys

sys.path.insert(0, "/opt/trn_rl_repo")

import numpy as np

N_NODES = 50000
D = 128
N_CORES = 8
WIN = 512           # dst slots per window (= PSUM bank: 512 f32/partition)
G = 4               # chunks per streamed SBUF tile (4KB/partition descriptors)
BUFS = 20           # stream tile pool depth
PF = 18             # prefetch tiles beyond current window's need
TSPLIT = 23         # split dst nodes with degree > TSPLIT across slots

_COMPILED = {}


def _preprocess(h, W, b, edges):
    import ml_dtypes
    bf16 = np.dtype(ml_dtypes.bfloat16)

    h = np.asarray(h, dtype=np.float32)
    W = np.asarray(W, dtype=np.float32)
    b = np.asarray(b, dtype=np.float32)
    loops = np.arange(N_NODES, dtype=np.int64)
    src = np.concatenate([np.asarray(edges[0], dtype=np.int64), loops])
    dst = np.concatenate([np.asarray(edges[1], dtype=np.int64), loops])

    deg = np.bincount(dst, minlength=N_NODES)  # >=1 (self loops)
    rs = deg.astype(np.float64) ** -0.5
    # wsumrs[d] = rs_d * sum_{e into d} rs_src   (bias coefficient)
    wsumrs = (np.bincount(dst, weights=rs[src], minlength=N_NODES) * rs
              ).astype(np.float32)

    # split high-degree dst nodes across several virtual slots (flattens the
    # per-round chunk template; host sums the partial columns afterwards)
    kparts = -(-deg // TSPLIT)                          # parts per node
    vbase = np.zeros(N_NODES + 1, np.int64)
    vbase[1:] = np.cumsum(kparts)
    nv = int(vbase[-1])
    vnode = np.repeat(np.arange(N_NODES), kparts)       # virtual -> node
    part_idx = np.arange(nv) - vbase[vnode]
    vdeg = deg[vnode] // kparts[vnode] + (part_idx < deg[vnode] % kparts[vnode])

    # degree-sorted windows of WIN slots over virtual nodes
    order = np.argsort(vdeg, kind="stable")
    NW = N_CORES * (-(-nv // (WIN * N_CORES)))          # windows (padded)
    slots_total = NW * WIN
    assert nv <= slots_total
    slot_of = np.empty(nv, np.int64)
    slot_of[order] = np.arange(nv)
    degs_p = np.zeros(slots_total, np.int64)
    degs_p[:nv] = vdeg[order]
    C_w = np.maximum(degs_p.reshape(NW, WIN).max(axis=1), 1)

    # deal windows to cores in rounds of 8, sorted by C desc; shared template
    worder = np.argsort(-C_w, kind="stable")
    NR = NW // N_CORES
    win_round = np.empty(NW, np.int64)
    win_core = np.empty(NW, np.int64)
    win_round[worder] = np.arange(NW) // N_CORES
    win_core[worder] = np.arange(NW) % N_CORES
    C_template = C_w[worder].reshape(NR, N_CORES).max(axis=1)
    off = np.zeros(NR + 1, np.int64)
    off[1:] = np.cumsum(C_template)
    NCH = int(off[-1])                                  # chunks per core
    NCHp = -(-NCH // G) * G

    # per-edge placement: (core, chunk, slot) via virtual dst slots
    es = np.argsort(dst, kind="stable")
    starts = np.searchsorted(dst[es], np.arange(N_NODES))
    rank = np.empty(dst.size, np.int64)
    rank[es] = np.arange(dst.size) - starts[dst[es]]
    kd = kparts[dst]
    vdst = vbase[dst] + rank % kd                       # virtual dst node
    vrank = rank // kd                                  # rank within part
    gslot = slot_of[vdst]
    w_e = gslot // WIN
    s_e = gslot % WIN
    j_e = win_round[w_e]
    c_e = win_core[w_e]
    col = (off[j_e] + vrank) * WIN + s_e                # column in core stream
    wgt = (rs[src] * rs[dst]).astype(np.float32)

    # per-slot metadata rows (wsumrs on part 0 + output node mapping)
    g_all = np.arange(slots_total)
    w_all = g_all // WIN
    pos_all = win_round[w_all] * WIN + (g_all % WIN)
    core_all = win_core[w_all]
    node_all = np.full(slots_total, -1, np.int64)
    node_all[:nv] = vnode[order]
    ws_all = np.zeros(slots_total, np.float32)
    ws_all[:nv] = np.where(part_idx[order] == 0, wsumrs[vnode[order]], 0.0)
    wrow = np.zeros((N_CORES, NR * WIN), np.float32)
    node_at = np.full((N_CORES, NR * WIN), -1, np.int64)
    wrow[core_all, pos_all] = ws_all
    node_at[core_all, pos_all] = node_all

    Wt = np.ascontiguousarray(W.T).astype(bf16)
    brow = b.reshape(1, D).astype(bf16)

    in_maps = []
    for c in range(N_CORES):
        m = c_e == c
        vals = (h[src[m]] * wgt[m][:, None]).astype(bf16)       # [E_c, 128]
        sarr = np.zeros((NCHp * WIN, D), bf16)
        sarr[col[m]] = vals
        in_maps.append({
            "stream": np.ascontiguousarray(sarr.T),             # [128, cols]
            "wsum": np.ascontiguousarray(wrow[c:c + 1]).astype(bf16),
            "Wt": Wt, "b": brow,
        })

    geom = dict(C_template=tuple(int(x) for x in C_template), NCHp=NCHp)
    return in_maps, node_at, geom


def _build_nc(geom):
    import concourse.bacc as bacc
    import concourse.mybir as mybir
    import concourse.tile as tile

    Ct = geom["C_template"]
    NR = len(Ct)
    NCHp = geom["NCHp"]
    NT = NCHp // G
    bf16, f32 = mybir.dt.bfloat16, mybir.dt.float32

    nc = bacc.Bacc("TRN2", target_bir_lowering=False, debug=False,
                   num_devices=N_CORES)
    stream_d = nc.declare_dram_parameter("stream", [D, NCHp * WIN], bf16,
                                         isOutput=False)
    wsum_d = nc.declare_dram_parameter("wsum", [1, NR * WIN], bf16,
                                       isOutput=False)
    Wt_d = nc.declare_dram_parameter("Wt", [D, D], bf16, isOutput=False)
    b_d = nc.declare_dram_parameter("b", [1, D], bf16, isOutput=False)
    out_d = nc.declare_dram_parameter("out", [D, NR * WIN], bf16, isOutput=True)

    with tile.TileContext(nc) as tc:
        with (
            tc.tile_pool(name="const", bufs=1) as cpool,
            tc.tile_pool(name="xs", bufs=BUFS) as xs,
            tc.tile_pool(name="wp", bufs=3) as wp,
            tc.tile_pool(name="ps", bufs=4, space="PSUM") as ps,
        ):
            Wt_t = cpool.tile([D, D], bf16)
            nc.sync.dma_start(out=Wt_t[:], in_=Wt_d[:])
            b_t = cpool.tile([1, D], bf16)
            nc.sync.dma_start(out=b_t[:], in_=b_d[:])
            wsum_t = cpool.tile([1, NR * WIN], bf16)
            nc.sync.dma_start(out=wsum_t[:], in_=wsum_d[:])

            tiles = [None] * NT
            issued = 0
            issuers = (nc.sync, nc.scalar)

            def issue():
                nonlocal issued
                t = xs.tile([D, G * WIN], bf16, tag="x")
                issuers[issued % len(issuers)].dma_start(
                    out=t[:],
                    in_=stream_d[:, issued * G * WIN:(issued + 1) * G * WIN])
                tiles[issued] = t
                issued += 1

            off = 0
            osb = None
            for j in range(NR):
                need = off + Ct[j]
                want = min(NT, -(-need // G) + PF)
                while issued < want:
                    issue()
                pacc = ps.tile([D, WIN], f32, tag="acc")
                nc.tensor.matmul(out=pacc[:], lhsT=b_t[:],
                                 rhs=wsum_t[:, j * WIN:(j + 1) * WIN],
                                 start=True, stop=False)
                for t in range(Ct[j]):
                    c = off + t
                    xt = tiles[c // G][:, (c % G) * WIN:(c % G + 1) * WIN]
                    nc.tensor.matmul(out=pacc[:], lhsT=Wt_t[:], rhs=xt,
                                     start=False, stop=(t == Ct[j] - 1))
                off = need
                osb = wp.tile([D, WIN], bf16, tag="o")
                nc.scalar.copy(out=osb[:], in_=pacc[:])
                nc.sync.dma_start(out=out_d[:, j * WIN:(j + 1) * WIN],
                                  in_=osb[:])

    nc.finalize()
    return nc


def _get_nc(geom):
    key = (geom["C_template"], geom["NCHp"])
    if key not in _COMPILED:
        _COMPILED[key] = _build_nc(geom)
    return _COMPILED[key]


def _assemble(res, node_at):
    out = np.zeros((N_NODES, D), np.float32)
    for c in range(N_CORES):
        valid = node_at[c] >= 0
        np.add.at(out, node_at[c][valid],
                  res.results[c]["out"][:, valid].T.astype(np.float32))
    return out


def kernel(h, W, b, edges):
    from concourse.bass_utils import run_bass_kernel_spmd

    in_maps, node_at, geom = _preprocess(h, W, b, edges)
    nc = _get_nc(geom)
    res = None
    last_exc = None
    for _attempt in range(3):
        try:
            res = run_bass_kernel_spmd(nc, in_maps, list(range(N_CORES)))
            break
        except Exception as e:  # transient axon/NRT hiccups
            last_exc = e
            import time
            time.sleep(2.0)
    if res is None:
        raise last_exc
    return _assemble(res, node_at)
